# revision 37
# baseline (speedup 1.0000x reference)
"""Trainium2 Bass kernel for ConstantTimeStrideAttention (CTSA).

Problem (hardcoded): B=2, S=4096, D=1536, H=12 heads, head dim d=128.
Each query s attends to 12 anchors: band offsets {+-1,+-2,+-3} (weight gw0),
{+-5,+-10} (weight gw1), and globals {0, S-1} (weight gw2 each), where
gw = softmax(group_scale).  softmax over the 12 anchor scores with additive
log-weights == multiplicative weights on exp(score).

Sharding: pure data parallel over (B=2) x (4 sequence chunks of 1024 rows)
-> 8 cores, no collectives.  Each core receives a 1056-row extended slice
of x (2 global rows + 14-left halo + 1024 own + 10-right halo + pad),
pre-transposed and cast to bf16 on the host.

On-core pipeline (bf16 on the PE, fp32 accumulation):
  1) v projection in natural layout [key, feat], with a ones column per
     head so the AV matmul also produces the softmax denominator.
     (v bias is folded into a host-side constant: sum_j P == 1.)
  2) per head h: q^T/k^T projection tiles via matmul(lhsT=W^T, rhs=x^T);
     K^T is written in a per-query-tile replicated "window" layout
     (8 slots x [160-wide window | 2 global cols]); then attention for
     the previous head (keeps dense GEMM work interleaved with the
     sparse attention matmuls so the PE HAM clock stays at 2.4 GHz).
  3) attention per (h, query-tile t): transposed scores
     S^T = matmul(lhsT=K^T window pieces, rhs=Q^T tile) -> one exp (ACT)
     -> one banded-weight mask multiply (DVE) -> A_nat & denominator in
     one accumulation group (rhs = V pieces with ones column), normalize
     with per-partition reciprocal, transpose via identity matmul -> A^T.
  4) out projection: Y^T = matmul(lhsT=Wo^T, rhs=A^T) -> fp32 out.
Host adds (b_v @ Wo^T + out_b) and stitches chunks together.
"""

import numpy as np
import ml_dtypes

import concourse.bass as bass
import concourse.mybir as mybir
import concourse.tile as tile
from concourse import bacc
from concourse.tile_autobufs import add_dep_helper
from concourse import bass_utils as _bu
from concourse.bass_utils import run_bass_kernel_spmd

del _bu  # (walrus --enable-ldw-opt=true breaks codegen; keep default)

BF16 = mybir.dt.bfloat16
F32 = mybir.dt.float32

B, S, D = 2, 4096, 1536
H, d = 12, 128
N_CORES = 8
CHUNK = 1024          # own rows per core
XROWS = 1056          # extended rows: 2 glob + 14 halo + 1024 + 10 halo + 6 pad
OWN0 = 16             # first own row inside x_ext
WIN = 160             # window width (keys) per query tile
SLOT = 162            # window + 2 global columns
NT = 8                # query tiles per core
VS = 129              # per-(tile,head) V slot width: 128 features + ones col
ALPHA = float(d) ** -0.5

_prog_cache = {}


def _build_program():
    if "nc" in _prog_cache:
        return _prog_cache["nc"]

    nc = bacc.Bacc(
        "TRN2", target_bir_lowering=False, debug=False, num_devices=N_CORES)

    # all inputs pre-swizzled on the host into on-chip layouts so every
    # DMA reads contiguous memory (strided gathers measured ~208 GB/s)
    xT_d = nc.dram_tensor("xT", [128, D // 128, XROWS], BF16,
                          kind="ExternalInput")
    wqk_d = nc.dram_tensor("wqk", [24, 128, D // 128, 128], BF16,
                           kind="ExternalInput")
    wv_d = nc.dram_tensor("wv", [3, 128, D // 128, 512], BF16,
                          kind="ExternalInput")
    wo_d = nc.dram_tensor("wo", [12, 128, D // 128, 128], BF16,
                          kind="ExternalInput")
    qkbias_d = nc.dram_tensor("qkbias", [128, 24], F32, kind="ExternalInput")
    wmask_d = nc.dram_tensor("wmask", [128, 3, 256], BF16, kind="ExternalInput")
    ident_d = nc.dram_tensor("ident", [128, 128], BF16, kind="ExternalInput")
    yT_d = nc.dram_tensor("yT", [D, CHUNK], F32, kind="ExternalOutput")

    KO = D // 128  # 12 k-tiles along the contraction dim
    ident_fn = mybir.ActivationFunctionType.Identity
    exp_fn = mybir.ActivationFunctionType.Exp

    with tile.TileContext(nc) as tc:
        with (
            tc.tile_pool(name="persist", bufs=1) as persist,
            tc.tile_pool(name="wq", bufs=2) as wqp,
            tc.tile_pool(name="wv", bufs=2) as wvp,
            tc.tile_pool(name="wo", bufs=3) as wop,
            tc.tile_pool(name="work", bufs=4) as work,
            tc.tile_pool(name="yst", bufs=3) as yst,
            tc.tile_pool(name="proj_ps", bufs=2, space="PSUM") as proj_ps,
            tc.tile_pool(name="p3_ps", bufs=1, space="PSUM") as p3_ps,
            tc.tile_pool(name="sc_ps", bufs=3, space="PSUM") as sc_ps,
            tc.tile_pool(name="ad_ps", bufs=2, space="PSUM") as ad_ps,
        ):
            # ---------- persistent SBUF tensors ----------
            xT = persist.tile([128, KO, XROWS], BF16)
            for kg in range(4):  # split so the PE can start sooner
                nc.sync.dma_start(xT[:, 3 * kg:3 * (kg + 1), :],
                                  xT_d[:, 3 * kg:3 * (kg + 1), :])

            qkbias = persist.tile([128, 24], F32)
            nc.gpsimd.dma_start(qkbias[:], qkbias_d[:])
            wmask = persist.tile([128, 3, 256], BF16)
            nc.gpsimd.dma_start(wmask[:], wmask_d[:])
            ident = persist.tile([128, 128], BF16)
            nc.gpsimd.dma_start(ident[:], ident_d[:])

            QT = persist.tile([128, H, CHUNK], BF16)       # Q^T, s in [16,1040)
            KTw = persist.tile([128, H, NT * SLOT], BF16)  # K^T windows
            V = persist.tile([128, NT, H, VS], BF16)       # V natural + ones col
            Vtail = persist.tile([34, NT, H, VS], BF16)    # 32 tail rows + 2 glob
            Vglob = persist.tile([2, D], BF16)
            AT = persist.tile([128, H, CHUNK], BF16)       # attention out ^T

            nc.gpsimd.memset(V[:, :, :, 128:129], 1.0)
            nc.gpsimd.memset(Vtail[:, :, :, 128:129], 1.0)

            # ---------- phase 1: v projection (natural layout) ----------
            # Weight DMAs are staggered behind compute progress so startup
            # HBM bandwidth all goes to x^T (otherwise the PE idles ~20us).
            gate_insts = {}
            for fc in range(3):
                wv = wvp.tile([128, KO, 512], BF16, tag="wv")
                dma = nc.sync.dma_start(wv[:], wv_d[fc])
                if fc >= 1 and (fc - 1, 3) in gate_insts:
                    add_dep_helper(dma.ins, gate_insts[(fc - 1, 3)].ins,
                                   sync=True, reason="stagger wv dma")
                for st in range(9):
                    rows = 128 if st < 8 else 32
                    ps = proj_ps.tile([128, 512], F32, tag="pps")
                    for kt in range(KO):
                        nc.tensor.matmul(
                            ps[0:rows, :],
                            xT[:, kt, st * 128: st * 128 + rows], wv[:, kt, :],
                            start=(kt == 0), stop=(kt == KO - 1),
                        )
                    psv = ps.rearrange("p (h f) -> p h f", f=128)
                    if st < 8:
                        cp = nc.vector.tensor_copy(
                            V[:, st, 4 * fc:4 * fc + 4, 0:128], psv[:])
                        gate_insts[(fc, st)] = cp
                    if 1 <= st <= 8:
                        nc.vector.tensor_copy(
                            Vtail[0:32, st - 1, 4 * fc:4 * fc + 4, 0:128],
                            psv[0:32])
                    if st == 0:
                        nc.vector.tensor_copy(
                            Vglob[:, fc * 512:(fc + 1) * 512], ps[0:2, :])
            # replicate global v rows into every tail slot (partition shift -> DMA)
            vgv = Vglob.rearrange("p (h f) -> p h f", f=128)
            for t in range(NT):
                nc.sync.dma_start(Vtail[32:34, t, :, 0:128], vgv[:])

            # ---------- phase 2+3: per-head qk projection + attention ----------
            def qk_proj(h):
                # q section (f-tile h): own rows only, s in [16, 1040)
                w = wqp.tile([128, KO, 128], BF16, tag="wq")
                dma = nc.sync.dma_start(w[:], wqk_d[h])
                if h == 0:
                    add_dep_helper(dma.ins, gate_insts[(0, 6)].ins,
                                   sync=True, reason="stagger wq dma")
                for ncl in range(2):
                    ps = proj_ps.tile([128, 512], F32, tag="pps")
                    for kt in range(KO):
                        nc.tensor.matmul(
                            ps[:], w[:, kt, :],
                            xT[:, kt, OWN0 + ncl * 512: OWN0 + (ncl + 1) * 512],
                            start=(kt == 0), stop=(kt == KO - 1),
                        )
                    # QT = (ps + bias) * alpha, on DVE
                    nc.vector.tensor_scalar(
                        QT[:, h, ncl * 512:(ncl + 1) * 512], ps[:],
                        qkbias[:, h:h + 1], ALPHA,
                        mybir.AluOpType.add, mybir.AluOpType.mult,
                    )
                # k section (f-tile 12+h): full extended rows, windowed layout
                ft = 12 + h
                w2 = wqp.tile([128, KO, 128], BF16, tag="wq")
                nc.sync.dma_start(w2[:], wqk_d[ft])
                ktw = KTw[:, h, :].rearrange("p (t j) -> p t j", j=SLOT)
                bias = qkbias[:, ft:ft + 1]
                ps3 = p3_ps.tile([128, 32], F32, tag="p3")
                for ncl in range(2):
                    ps = proj_ps.tile([128, 512], F32, tag="pps")
                    for kt in range(KO):
                        nc.tensor.matmul(
                            ps[:], w2[:, kt, :],
                            xT[:, kt, ncl * 512:(ncl + 1) * 512],
                            start=(kt == 0), stop=(kt == KO - 1),
                        )
                        if ncl == 1:
                            # keys 1024..1056: same weights — its LDWEIGHTS
                            # hides under the 512-wide stream above
                            nc.tensor.matmul(
                                ps3[:], w2[:, kt, :], xT[:, kt, 1024:1056],
                                start=(kt == 0), stop=(kt == KO - 1),
                            )
                    psv = ps.rearrange("p (t j) -> p t j", j=128)
                    t0 = 4 * ncl
                    nc.scalar.activation(
                        ktw[:, t0:t0 + 4, 0:128], psv[:, 0:4, :],
                        ident_fn, bias=bias)
                    if ncl == 0:
                        nc.scalar.activation(
                            ktw[:, 0:3, 128:160], psv[:, 1:4, 0:32],
                            ident_fn, bias=bias)
                        nc.scalar.activation(
                            ktw[:, 0:NT, 160:162],
                            ps[:, None, 0:2].to_broadcast([128, NT, 2]),
                            ident_fn, bias=bias)
                    else:
                        nc.scalar.activation(
                            ktw[:, 3:7, 128:160], psv[:, 0:4, 0:32],
                            ident_fn, bias=bias)
                        nc.scalar.activation(
                            ktw[:, 7:8, 128:160], ps3[:, None, 0:32],
                            ident_fn, bias=bias)

            def attention(h):
                ktw = KTw[:, h, :].rearrange("p (t j) -> p t j", j=SLOT)
                for t in range(NT):
                    m = 0 if t == 0 else (2 if t == NT - 1 else 1)
                    qt = QT[:, h, t * 128:(t + 1) * 128]
                    sc = sc_ps.tile([128, 256], F32, tag="sc")
                    nc.tensor.matmul(sc[:, 0:128], ktw[:, t, 0:128], qt,
                                     start=True, stop=True)
                    nc.tensor.matmul(sc[0:34, 128:256], ktw[:, t, 128:162], qt,
                                     start=True, stop=True)
                    pe = work.tile([128, 256], BF16, tag="pe")
                    nc.scalar.activation(pe[:], sc[:], exp_fn)
                    pm = work.tile([128, 256], BF16, tag="pm")
                    nc.vector.tensor_mul(pm[:], pe[:], wmask[:, m, :])

                    ad = ad_ps.tile([128, 260], F32, tag="ad")
                    nc.tensor.matmul(ad[:, 0:VS], pm[:, 0:128],
                                     V[:, t, h, :], start=True, stop=False)
                    nc.tensor.matmul(ad[:, 0:VS], pm[0:34, 128:256],
                                     Vtail[:, t, h, :], start=False, stop=True)

                    r = work.tile([128, 1], F32, tag="r")
                    nc.vector.reciprocal(r[:], ad[:, 128:129])
                    a_sb = work.tile([128, 128], BF16, tag="a_sb")
                    nc.vector.tensor_scalar_mul(a_sb[:], ad[:, 0:128], r[:])
                    # transpose: A^T = a_sb.T @ I
                    nc.tensor.matmul(ad[:, 132:260], a_sb[:], ident[:],
                                     start=True, stop=True)
                    nc.vector.tensor_copy(AT[:, h, t * 128:(t + 1) * 128],
                                          ad[:, 132:260])

            qk_proj(0)
            for h in range(1, H):
                qk_proj(h)
                attention(h - 1)
            attention(H - 1)

            # ---------- phase 4: out projection ----------
            for ft in range(12):
                wo = wop.tile([128, KO, 128], BF16, tag="wo")
                nc.sync.dma_start(wo[:], wo_d[ft])
                for ncl in range(2):
                    ps = proj_ps.tile([128, 512], F32, tag="pps")
                    for kt in range(KO):
                        nc.tensor.matmul(
                            ps[:], wo[:, kt, :], AT[:, kt, ncl * 512:(ncl + 1) * 512],
                            start=(kt == 0), stop=(kt == KO - 1),
                        )
                    y = yst.tile([128, 512], F32, tag="y")
                    nc.scalar.copy(y[:], ps[:])
                    nc.sync.dma_start(
                        yT_d.rearrange("(fo p) s -> p fo s", p=128)
                        [:, ft, ncl * 512:(ncl + 1) * 512], y[:])

    nc.compile()
    _prog_cache["nc"] = nc
    return nc


def _host_prep(x, qkv_w, qkv_b, out_w, out_b, group_scale):
    """Build the per-core input maps (numpy only)."""
    bf16 = ml_dtypes.bfloat16
    g = np.asarray(group_scale, np.float64)
    e = np.exp(g - g.max())
    gw = (e / e.sum()).astype(np.float64)

    KO = D // 128
    wT = qkv_w.astype(np.float32).T              # [D, 3D]
    # q/k sections, f-tile major: [24, 128, KO, 128]
    wqk = np.ascontiguousarray(
        wT[:, :2 * D].reshape(KO, 128, 24, 128).transpose(2, 1, 0, 3)
    ).astype(bf16)
    # v section, 512-wide f-chunk major: [3, 128, KO, 512]
    wv = np.ascontiguousarray(
        wT[:, 2 * D:].reshape(KO, 128, 3, 512).transpose(2, 1, 0, 3)
    ).astype(bf16)
    woT = out_w.astype(np.float32).T             # [D, D]
    wo = np.ascontiguousarray(
        woT.reshape(KO, 128, 12, 128).transpose(2, 1, 0, 3)
    ).astype(bf16)

    qkbias = np.zeros((128, 24), np.float32)
    for ft in range(24):
        qkbias[:, ft] = qkv_b[ft * 128:(ft + 1) * 128].astype(np.float32)

    ident = np.eye(128, dtype=bf16)

    band = [(-1, 0), (1, 0), (-2, 0), (2, 0), (-3, 0), (3, 0),
            (-5, 1), (5, 1), (-10, 1), (10, 1)]

    in_maps = []
    for core in range(N_CORES):
        b, chunk = divmod(core, 4)
        c0 = chunk * CHUNK
        xe = np.zeros((XROWS, D), np.float32)
        xe[0] = x[b, 0]
        xe[1] = x[b, S - 1]
        if chunk > 0:
            xe[2:16] = x[b, c0 - 14:c0]
        xe[16:16 + CHUNK] = x[b, c0:c0 + CHUNK]
        if chunk < 3:
            xe[16 + CHUNK:26 + CHUNK] = x[b, c0 + CHUNK:c0 + CHUNK + 10]
        xT = np.ascontiguousarray(
            xe.T.reshape(KO, 128, XROWS).transpose(1, 0, 2)).astype(bf16)

        # combined banded weight mask, [j, slot, 256]:
        #   cols 0:128 -> window piece a (keys 128t..128t+128)
        #   cols 128:256 rows 0:32 -> tail keys, rows 32:34 -> globals
        wm = np.zeros((128, 3, 256), np.float64)
        for slot, t in ((0, 0), (1, 3), (2, NT - 1)):
            for p in range(128):
                s = c0 + 128 * t + p
                for off, grp in band:
                    a = min(max(s + off, 0), S - 1)
                    j = (a - c0 + 16) - 128 * t
                    if j < 128:
                        wm[j, slot, p] += gw[grp]
                    else:
                        wm[j - 128, slot, 128 + p] += gw[grp]
            wm[32, slot, 128:256] += gw[2]
            wm[33, slot, 128:256] += gw[2]

        in_maps.append({
            "xT": xT,
            "wqk": wqk,
            "wv": wv,
            "wo": wo,
            "qkbias": qkbias,
            "wmask": wm.astype(bf16),
            "ident": ident,
        })

    y_const = (qkv_b[2 * D:3 * D].astype(np.float64) @
               out_w.astype(np.float64).T + out_b.astype(np.float64)
               ).astype(np.float32)
    return in_maps, y_const


def kernel(x, qkv_w, qkv_b, out_w, out_b, group_scale, _run_kwargs=None):
    x = np.asarray(x)
    in_maps, y_const = _host_prep(
        np.asarray(x, np.float32), np.asarray(qkv_w, np.float32),
        np.asarray(qkv_b, np.float32), np.asarray(out_w, np.float32),
        np.asarray(out_b, np.float32), np.asarray(group_scale, np.float32))
    nc = _build_program()
    kwargs = _run_kwargs or {}
    res = run_bass_kernel_spmd(nc, in_maps, core_ids=list(range(N_CORES)), **kwargs)
    out = np.empty((B, S, D), np.float32)
    for core in range(N_CORES):
        b, chunk = divmod(core, 4)
        r = res.results[core]
        yT = r["yT"] if isinstance(r, dict) else r
        out[b, chunk * CHUNK:(chunk + 1) * CHUNK] = np.asarray(yT, np.float32).T
    out += y_const
    if kwargs.get("trace"):
        kernel.last_exec_time_ns = res.exec_time_ns
    return out


if __name__ == "__main__":
    rng = np.random.default_rng(0)
    x = rng.standard_normal((B, S, D), dtype=np.float32)
    qkv_w = (rng.standard_normal((3 * D, D), dtype=np.float32) / np.sqrt(D))
    qkv_b = rng.standard_normal(3 * D, dtype=np.float32) * 0.01
    out_w = rng.standard_normal((D, D), dtype=np.float32) / np.sqrt(D)
    out_b = rng.standard_normal(D, dtype=np.float32) * 0.01
    gs = rng.standard_normal(3, dtype=np.float32)
    y = kernel(x=x, qkv_w=qkv_w, qkv_b=qkv_b, out_w=out_w, out_b=out_b,
               group_scale=gs)
    print("ok", y.shape, float(np.abs(y).mean()))


# revision 39
# speedup vs baseline: 1.1559x; 1.1559x over previous
"""Trainium2 Bass kernel for ConstantTimeStrideAttention (CTSA).

Problem (hardcoded): B=2, S=4096, D=1536, H=12 heads, head dim d=128.
Each query s attends to 12 anchors: band offsets {+-1,+-2,+-3} (weight gw0),
{+-5,+-10} (weight gw1), and globals {0, S-1} (weight gw2 each), where
gw = softmax(group_scale).  softmax over the 12 anchor scores with additive
log-weights == multiplicative weights on exp(score).

Sharding: pure data parallel over (B=2) x (4 sequence chunks of 1024 rows)
-> 8 cores, no collectives.  Each core receives a 1056-row extended slice
of x (2 global rows + 14-left halo + 1024 own + 10-right halo + pad),
pre-transposed and cast to bf16 on the host.

On-core pipeline (bf16 on the PE, fp32 accumulation):
  1) v projection in natural layout [key, feat], with a ones column per
     head so the AV matmul also produces the softmax denominator.
     (v bias is folded into a host-side constant: sum_j P == 1.)
  2) per head h: q^T/k^T projection tiles via matmul(lhsT=W^T, rhs=x^T);
     K^T is written in a per-query-tile replicated "window" layout
     (8 slots x [160-wide window | 2 global cols]); then attention for
     the previous head (keeps dense GEMM work interleaved with the
     sparse attention matmuls so the PE HAM clock stays at 2.4 GHz).
  3) attention per (h, query-tile t): transposed scores
     S^T = matmul(lhsT=K^T window pieces, rhs=Q^T tile) -> one exp (ACT)
     -> one banded-weight mask multiply (DVE) -> A_nat & denominator in
     one accumulation group (rhs = V pieces with ones column), normalize
     with per-partition reciprocal, transpose via identity matmul -> A^T.
  4) out projection: Y^T = matmul(lhsT=Wo^T, rhs=A^T) -> fp32 out.
Host adds (b_v @ Wo^T + out_b) and stitches chunks together.
"""

import numpy as np
import ml_dtypes

import concourse.bass as bass
import concourse.mybir as mybir
import concourse.tile as tile
from concourse import bacc
from concourse.tile_autobufs import add_dep_helper
from concourse import bass_utils as _bu
from concourse.bass_utils import run_bass_kernel_spmd

del _bu  # (walrus --enable-ldw-opt=true breaks codegen; keep default)

BF16 = mybir.dt.bfloat16
F32 = mybir.dt.float32

B, S, D = 2, 4096, 1536
H, d = 12, 128
N_CORES = 8
CHUNK = 1024          # own rows per core
XROWS = 1056          # extended rows: 2 glob + 14 halo + 1024 + 10 halo + 6 pad
OWN0 = 16             # first own row inside x_ext
WIN = 160             # window width (keys) per query tile
SLOT = 162            # window + 2 global columns
NT = 8                # query tiles per core
VS = 129              # per-(tile,head) V slot width: 128 features + ones col
ALPHA = float(d) ** -0.5

_prog_cache = {}


def _build_program():
    if "nc" in _prog_cache:
        return _prog_cache["nc"]

    nc = bacc.Bacc(
        "TRN2", target_bir_lowering=False, debug=False, num_devices=N_CORES)

    # all inputs pre-swizzled on the host into on-chip layouts so every
    # DMA reads contiguous memory (strided gathers measured ~208 GB/s)
    xT_d = nc.dram_tensor("xT", [128, D // 128, XROWS], BF16,
                          kind="ExternalInput")
    wqk_d = nc.dram_tensor("wqk", [24, 128, D // 128, 128], BF16,
                           kind="ExternalInput")
    wv_d = nc.dram_tensor("wv", [3, 128, D // 128, 512], BF16,
                          kind="ExternalInput")
    wo_d = nc.dram_tensor("wo", [12, 128, D // 128, 128], BF16,
                          kind="ExternalInput")
    qkbias_d = nc.dram_tensor("qkbias", [128, 24], F32, kind="ExternalInput")
    wmask_d = nc.dram_tensor("wmask", [128, 3, 256], BF16, kind="ExternalInput")
    ident_d = nc.dram_tensor("ident", [128, 128], BF16, kind="ExternalInput")
    yT_d = nc.dram_tensor("yT", [D, CHUNK], F32, kind="ExternalOutput")

    KO = D // 128  # 12 k-tiles along the contraction dim
    ident_fn = mybir.ActivationFunctionType.Identity
    exp_fn = mybir.ActivationFunctionType.Exp

    with tile.TileContext(nc) as tc:
        with (
            tc.tile_pool(name="persist", bufs=1) as persist,
            tc.tile_pool(name="wq", bufs=2) as wqp,
            tc.tile_pool(name="wv", bufs=2) as wvp,
            tc.tile_pool(name="wo", bufs=3) as wop,
            tc.tile_pool(name="work", bufs=4) as work,
            tc.tile_pool(name="yst", bufs=3) as yst,
            tc.tile_pool(name="proj_ps", bufs=2, space="PSUM") as proj_ps,
            tc.tile_pool(name="p3_ps", bufs=1, space="PSUM") as p3_ps,
            tc.tile_pool(name="sc_ps", bufs=2, space="PSUM") as sc_ps,
            tc.tile_pool(name="ad_ps", bufs=3, space="PSUM") as ad_ps,
        ):
            # ---------- persistent SBUF tensors ----------
            xT = persist.tile([128, KO, XROWS], BF16)
            for kg in range(4):  # split so the PE can start sooner
                nc.sync.dma_start(xT[:, 3 * kg:3 * (kg + 1), :],
                                  xT_d[:, 3 * kg:3 * (kg + 1), :])

            qkbias = persist.tile([128, 24], F32)
            nc.gpsimd.dma_start(qkbias[:], qkbias_d[:])
            wmask = persist.tile([128, 3, 256], BF16)
            nc.gpsimd.dma_start(wmask[:], wmask_d[:])
            ident = persist.tile([128, 128], BF16)
            nc.gpsimd.dma_start(ident[:], ident_d[:])

            QT = persist.tile([128, H, CHUNK], BF16)       # Q^T, s in [16,1040)
            KTw = persist.tile([128, H, NT * SLOT], BF16)  # K^T windows
            V = persist.tile([128, NT, H, VS], BF16)       # V natural + ones col
            Vtail = persist.tile([34, NT, H, VS], BF16)    # 32 tail rows + 2 glob
            Vglob = persist.tile([2, D], BF16)
            AT = persist.tile([128, H, CHUNK], BF16)       # attention out ^T

            nc.gpsimd.memset(V[:, :, :, 128:129], 1.0)
            nc.gpsimd.memset(Vtail[:, :, :, 128:129], 1.0)

            # ---------- phase 1: v projection (natural layout) ----------
            # Weight DMAs are staggered behind compute progress so startup
            # HBM bandwidth all goes to x^T (otherwise the PE idles ~20us).
            gate_insts = {}
            for fc in range(3):
                wv = wvp.tile([128, KO, 512], BF16, tag="wv")
                dma = nc.sync.dma_start(wv[:], wv_d[fc])
                if fc >= 1 and (fc - 1, 3) in gate_insts:
                    add_dep_helper(dma.ins, gate_insts[(fc - 1, 3)].ins,
                                   sync=True, reason="stagger wv dma")
                for st in range(9):
                    rows = 128 if st < 8 else 32
                    ps = proj_ps.tile([128, 512], F32, tag="pps")
                    for kt in range(KO):
                        nc.tensor.matmul(
                            ps[0:rows, :],
                            xT[:, kt, st * 128: st * 128 + rows], wv[:, kt, :],
                            start=(kt == 0), stop=(kt == KO - 1),
                        )
                    psv = ps.rearrange("p (h f) -> p h f", f=128)
                    if st < 8:
                        cp = nc.vector.tensor_copy(
                            V[:, st, 4 * fc:4 * fc + 4, 0:128], psv[:])
                        gate_insts[(fc, st)] = cp
                    if 1 <= st <= 8:
                        nc.vector.tensor_copy(
                            Vtail[0:32, st - 1, 4 * fc:4 * fc + 4, 0:128],
                            psv[0:32])
                    if st == 0:
                        nc.vector.tensor_copy(
                            Vglob[:, fc * 512:(fc + 1) * 512], ps[0:2, :])
            # replicate global v rows into every tail slot (partition shift -> DMA)
            vgv = Vglob.rearrange("p (h f) -> p h f", f=128)
            for t in range(NT):
                nc.sync.dma_start(Vtail[32:34, t, :, 0:128], vgv[:])

            # ---------- phase 2+3: per-head qk projection + attention ----------
            def qk_proj(h):
                # q section (f-tile h): own rows only, s in [16, 1040)
                w = wqp.tile([128, KO, 128], BF16, tag="wq")
                dma = nc.sync.dma_start(w[:], wqk_d[h])
                if h == 0:
                    add_dep_helper(dma.ins, gate_insts[(0, 6)].ins,
                                   sync=True, reason="stagger wq dma")
                for ncl in range(2):
                    ps = proj_ps.tile([128, 512], F32, tag="pps")
                    for kt in range(KO):
                        nc.tensor.matmul(
                            ps[:], w[:, kt, :],
                            xT[:, kt, OWN0 + ncl * 512: OWN0 + (ncl + 1) * 512],
                            start=(kt == 0), stop=(kt == KO - 1),
                        )
                    # QT = (ps + bias) * alpha, on DVE
                    nc.vector.tensor_scalar(
                        QT[:, h, ncl * 512:(ncl + 1) * 512], ps[:],
                        qkbias[:, h:h + 1], ALPHA,
                        mybir.AluOpType.add, mybir.AluOpType.mult,
                    )
                # k section (f-tile 12+h): full extended rows, windowed layout
                ft = 12 + h
                w2 = wqp.tile([128, KO, 128], BF16, tag="wq")
                nc.sync.dma_start(w2[:], wqk_d[ft])
                ktw = KTw[:, h, :].rearrange("p (t j) -> p t j", j=SLOT)
                bias = qkbias[:, ft:ft + 1]
                ps3 = p3_ps.tile([128, 32], F32, tag="p3")
                for ncl in range(2):
                    ps = proj_ps.tile([128, 512], F32, tag="pps")
                    for kt in range(KO):
                        nc.tensor.matmul(
                            ps[:], w2[:, kt, :],
                            xT[:, kt, ncl * 512:(ncl + 1) * 512],
                            start=(kt == 0), stop=(kt == KO - 1),
                        )
                        if ncl == 1:
                            # keys 1024..1056: same weights — its LDWEIGHTS
                            # hides under the 512-wide stream above
                            nc.tensor.matmul(
                                ps3[:], w2[:, kt, :], xT[:, kt, 1024:1056],
                                start=(kt == 0), stop=(kt == KO - 1),
                            )
                    psv = ps.rearrange("p (t j) -> p t j", j=128)
                    t0 = 4 * ncl
                    nc.scalar.activation(
                        ktw[:, t0:t0 + 4, 0:128], psv[:, 0:4, :],
                        ident_fn, bias=bias)
                    if ncl == 0:
                        nc.scalar.activation(
                            ktw[:, 0:3, 128:160], psv[:, 1:4, 0:32],
                            ident_fn, bias=bias)
                        nc.scalar.activation(
                            ktw[:, 0:NT, 160:162],
                            ps[:, None, 0:2].to_broadcast([128, NT, 2]),
                            ident_fn, bias=bias)
                    else:
                        nc.scalar.activation(
                            ktw[:, 3:7, 128:160], psv[:, 0:4, 0:32],
                            ident_fn, bias=bias)
                        nc.scalar.activation(
                            ktw[:, 7:8, 128:160], ps3[:, None, 0:32],
                            ident_fn, bias=bias)

            def attention(h):
                ktw = KTw[:, h, :].rearrange("p (t j) -> p t j", j=SLOT)
                for t in range(NT):
                    m = 0 if t == 0 else (2 if t == NT - 1 else 1)
                    qt = QT[:, h, t * 128:(t + 1) * 128]
                    sc = sc_ps.tile([128, 256], F32, tag="sc")
                    nc.tensor.matmul(sc[:, 0:128], ktw[:, t, 0:128], qt,
                                     start=True, stop=True)
                    nc.tensor.matmul(sc[0:34, 128:256], ktw[:, t, 128:162], qt,
                                     start=True, stop=True)
                    pe = work.tile([128, 256], BF16, tag="pe")
                    nc.scalar.activation(pe[:], sc[:], exp_fn)
                    pm = work.tile([128, 256], BF16, tag="pm")
                    nc.vector.tensor_mul(pm[:], pe[:], wmask[:, m, :])

                    ad = ad_ps.tile([128, VS], F32, tag="ad")
                    nc.tensor.matmul(ad[:, 0:VS], pm[:, 0:128],
                                     V[:, t, h, :], start=True, stop=False)
                    nc.tensor.matmul(ad[:, 0:VS], pm[0:34, 128:256],
                                     Vtail[:, t, h, :], start=False, stop=True)

                    r = work.tile([128, 1], F32, tag="r")
                    nc.vector.reciprocal(r[:], ad[:, 128:129])
                    a_sb = work.tile([128, 128], BF16, tag="a_sb")
                    nc.vector.tensor_scalar_mul(a_sb[:], ad[:, 0:128], r[:])
                    # transpose: A^T = a_sb.T @ I (reuse the dead score psum)
                    nc.tensor.matmul(sc[:, 0:128], a_sb[:], ident[:],
                                     start=True, stop=True)
                    nc.vector.tensor_copy(AT[:, h, t * 128:(t + 1) * 128],
                                          sc[:, 0:128])

            qk_proj(0)
            for h in range(1, H):
                qk_proj(h)
                attention(h - 1)
            attention(H - 1)

            # ---------- phase 4: out projection ----------
            for ft in range(12):
                wo = wop.tile([128, KO, 128], BF16, tag="wo")
                nc.sync.dma_start(wo[:], wo_d[ft])
                for ncl in range(2):
                    ps = proj_ps.tile([128, 512], F32, tag="pps")
                    for kt in range(KO):
                        nc.tensor.matmul(
                            ps[:], wo[:, kt, :], AT[:, kt, ncl * 512:(ncl + 1) * 512],
                            start=(kt == 0), stop=(kt == KO - 1),
                        )
                    y = yst.tile([128, 512], F32, tag="y")
                    nc.scalar.copy(y[:], ps[:])
                    nc.sync.dma_start(
                        yT_d.rearrange("(fo p) s -> p fo s", p=128)
                        [:, ft, ncl * 512:(ncl + 1) * 512], y[:])

    nc.compile()
    _prog_cache["nc"] = nc
    return nc


def _host_prep(x, qkv_w, qkv_b, out_w, out_b, group_scale):
    """Build the per-core input maps (numpy only)."""
    bf16 = ml_dtypes.bfloat16
    g = np.asarray(group_scale, np.float64)
    e = np.exp(g - g.max())
    gw = (e / e.sum()).astype(np.float64)

    KO = D // 128
    wT = qkv_w.astype(np.float32).T              # [D, 3D]
    # q/k sections, f-tile major: [24, 128, KO, 128]
    wqk = np.ascontiguousarray(
        wT[:, :2 * D].reshape(KO, 128, 24, 128).transpose(2, 1, 0, 3)
    ).astype(bf16)
    # v section, 512-wide f-chunk major: [3, 128, KO, 512]
    wv = np.ascontiguousarray(
        wT[:, 2 * D:].reshape(KO, 128, 3, 512).transpose(2, 1, 0, 3)
    ).astype(bf16)
    woT = out_w.astype(np.float32).T             # [D, D]
    wo = np.ascontiguousarray(
        woT.reshape(KO, 128, 12, 128).transpose(2, 1, 0, 3)
    ).astype(bf16)

    qkbias = np.zeros((128, 24), np.float32)
    for ft in range(24):
        qkbias[:, ft] = qkv_b[ft * 128:(ft + 1) * 128].astype(np.float32)

    ident = np.eye(128, dtype=bf16)

    band = [(-1, 0), (1, 0), (-2, 0), (2, 0), (-3, 0), (3, 0),
            (-5, 1), (5, 1), (-10, 1), (10, 1)]

    in_maps = []
    for core in range(N_CORES):
        b, chunk = divmod(core, 4)
        c0 = chunk * CHUNK
        xe = np.zeros((XROWS, D), np.float32)
        xe[0] = x[b, 0]
        xe[1] = x[b, S - 1]
        if chunk > 0:
            xe[2:16] = x[b, c0 - 14:c0]
        xe[16:16 + CHUNK] = x[b, c0:c0 + CHUNK]
        if chunk < 3:
            xe[16 + CHUNK:26 + CHUNK] = x[b, c0 + CHUNK:c0 + CHUNK + 10]
        xT = np.ascontiguousarray(
            xe.T.reshape(KO, 128, XROWS).transpose(1, 0, 2)).astype(bf16)

        # combined banded weight mask, [j, slot, 256]:
        #   cols 0:128 -> window piece a (keys 128t..128t+128)
        #   cols 128:256 rows 0:32 -> tail keys, rows 32:34 -> globals
        wm = np.zeros((128, 3, 256), np.float64)
        for slot, t in ((0, 0), (1, 3), (2, NT - 1)):
            for p in range(128):
                s = c0 + 128 * t + p
                for off, grp in band:
                    a = min(max(s + off, 0), S - 1)
                    j = (a - c0 + 16) - 128 * t
                    if j < 128:
                        wm[j, slot, p] += gw[grp]
                    else:
                        wm[j - 128, slot, 128 + p] += gw[grp]
            wm[32, slot, 128:256] += gw[2]
            wm[33, slot, 128:256] += gw[2]

        in_maps.append({
            "xT": xT,
            "wqk": wqk,
            "wv": wv,
            "wo": wo,
            "qkbias": qkbias,
            "wmask": wm.astype(bf16),
            "ident": ident,
        })

    y_const = (qkv_b[2 * D:3 * D].astype(np.float64) @
               out_w.astype(np.float64).T + out_b.astype(np.float64)
               ).astype(np.float32)
    return in_maps, y_const


def kernel(x, qkv_w, qkv_b, out_w, out_b, group_scale, _run_kwargs=None):
    x = np.asarray(x)
    in_maps, y_const = _host_prep(
        np.asarray(x, np.float32), np.asarray(qkv_w, np.float32),
        np.asarray(qkv_b, np.float32), np.asarray(out_w, np.float32),
        np.asarray(out_b, np.float32), np.asarray(group_scale, np.float32))
    nc = _build_program()
    kwargs = _run_kwargs or {}
    res = run_bass_kernel_spmd(nc, in_maps, core_ids=list(range(N_CORES)), **kwargs)
    out = np.empty((B, S, D), np.float32)
    for core in range(N_CORES):
        b, chunk = divmod(core, 4)
        r = res.results[core]
        yT = r["yT"] if isinstance(r, dict) else r
        out[b, chunk * CHUNK:(chunk + 1) * CHUNK] = np.asarray(yT, np.float32).T
    out += y_const
    if kwargs.get("trace"):
        kernel.last_exec_time_ns = res.exec_time_ns
    return out


if __name__ == "__main__":
    rng = np.random.default_rng(0)
    x = rng.standard_normal((B, S, D), dtype=np.float32)
    qkv_w = (rng.standard_normal((3 * D, D), dtype=np.float32) / np.sqrt(D))
    qkv_b = rng.standard_normal(3 * D, dtype=np.float32) * 0.01
    out_w = rng.standard_normal((D, D), dtype=np.float32) / np.sqrt(D)
    out_b = rng.standard_normal(D, dtype=np.float32) * 0.01
    gs = rng.standard_normal(3, dtype=np.float32)
    y = kernel(x=x, qkv_w=qkv_w, qkv_b=qkv_b, out_w=out_w, out_b=out_b,
               group_scale=gs)
    print("ok", y.shape, float(np.abs(y).mean()))


# revision 43
# speedup vs baseline: 1.1949x; 1.0337x over previous
"""Trainium2 Bass kernel for ConstantTimeStrideAttention (CTSA).

Problem (hardcoded): B=2, S=4096, D=1536, H=12 heads, head dim d=128.
Each query s attends to 12 anchors: band offsets {+-1,+-2,+-3} (weight gw0),
{+-5,+-10} (weight gw1), and globals {0, S-1} (weight gw2 each), where
gw = softmax(group_scale).  softmax over the 12 anchor scores with additive
log-weights == multiplicative weights on exp(score).

Sharding: pure data parallel over (B=2) x (4 sequence chunks of 1024 rows)
-> 8 cores, no collectives.  Each core receives a 1056-row extended slice
of x (2 global rows + 14-left halo + 1024 own + 10-right halo + pad),
pre-transposed and cast to bf16 on the host.

On-core pipeline (bf16 on the PE, fp32 accumulation):
  1) v projection in natural layout [key, feat], with a ones column per
     head so the AV matmul also produces the softmax denominator.
     (v bias is folded into a host-side constant: sum_j P == 1.)
  2) per head h: q^T/k^T projection tiles via matmul(lhsT=W^T, rhs=x^T);
     K^T is written in a per-query-tile replicated "window" layout
     (8 slots x [160-wide window | 2 global cols]); then attention for
     the previous head (keeps dense GEMM work interleaved with the
     sparse attention matmuls so the PE HAM clock stays at 2.4 GHz).
  3) attention per (h, query-tile t): transposed scores
     S^T = matmul(lhsT=K^T window pieces, rhs=Q^T tile) -> one exp (ACT)
     -> one banded-weight mask multiply (DVE) -> A_nat & denominator in
     one accumulation group (rhs = V pieces with ones column), normalize
     with per-partition reciprocal, transpose via identity matmul -> A^T.
  4) out projection: Y^T = matmul(lhsT=Wo^T, rhs=A^T) -> fp32 out.
Host adds (b_v @ Wo^T + out_b) and stitches chunks together.
"""

import numpy as np
import ml_dtypes

import concourse.bass as bass
import concourse.mybir as mybir
import concourse.tile as tile
from concourse import bacc
from concourse.tile_autobufs import add_dep_helper
from concourse import bass_utils as _bu
from concourse.bass_utils import run_bass_kernel_spmd

del _bu  # (walrus --enable-ldw-opt=true breaks codegen; keep default)

BF16 = mybir.dt.bfloat16
F32 = mybir.dt.float32

B, S, D = 2, 4096, 1536
H, d = 12, 128
N_CORES = 8
CHUNK = 1024          # own rows per core
XROWS = 1056          # extended rows: 2 glob + 14 halo + 1024 + 10 halo + 6 pad
OWN0 = 16             # first own row inside x_ext
WIN = 160             # window width (keys) per query tile
SLOT = 162            # window + 2 global columns
NT = 8                # query tiles per core
VS = 129              # per-(tile,head) V slot width: 128 features + ones col
ALPHA = float(d) ** -0.5

_prog_cache = {}


def _build_program():
    if "nc" in _prog_cache:
        return _prog_cache["nc"]

    nc = bacc.Bacc(
        "TRN2", target_bir_lowering=False, debug=False, num_devices=N_CORES)

    # all inputs pre-swizzled on the host into on-chip layouts so every
    # DMA reads contiguous memory (strided gathers measured ~208 GB/s)
    xT_d = nc.dram_tensor("xT", [128, D // 128, XROWS], BF16,
                          kind="ExternalInput")
    wqk_d = nc.dram_tensor("wqk", [24, 128, D // 128, 128], BF16,
                           kind="ExternalInput")
    wv_d = nc.dram_tensor("wv", [3, 128, D // 128, 512], BF16,
                          kind="ExternalInput")
    wo_d = nc.dram_tensor("wo", [12, 128, D // 128, 128], BF16,
                          kind="ExternalInput")
    qkbias_d = nc.dram_tensor("qkbias", [128, 24], F32, kind="ExternalInput")
    wmask_d = nc.dram_tensor("wmask", [128, 3, 256], BF16, kind="ExternalInput")
    ident_d = nc.dram_tensor("ident", [128, 128], BF16, kind="ExternalInput")
    yT_d = nc.dram_tensor("yT", [D, CHUNK], F32, kind="ExternalOutput")

    KO = D // 128  # 12 k-tiles along the contraction dim
    ident_fn = mybir.ActivationFunctionType.Identity
    exp_fn = mybir.ActivationFunctionType.Exp

    with tile.TileContext(nc) as tc:
        with (
            tc.tile_pool(name="persist", bufs=1) as persist,
            tc.tile_pool(name="wq", bufs=3) as wqp,
            tc.tile_pool(name="wv", bufs=2) as wvp,
            tc.tile_pool(name="wo", bufs=3) as wop,
            tc.tile_pool(name="work", bufs=4) as work,
            tc.tile_pool(name="yst", bufs=2) as yst,
            tc.tile_pool(name="proj_ps", bufs=2, space="PSUM") as proj_ps,
            tc.tile_pool(name="p3_ps", bufs=1, space="PSUM") as p3_ps,
            tc.tile_pool(name="sc_ps", bufs=2, space="PSUM") as sc_ps,
            tc.tile_pool(name="ad_ps", bufs=3, space="PSUM") as ad_ps,
        ):
            # ---------- persistent SBUF tensors ----------
            xT = persist.tile([128, KO, XROWS], BF16)
            for kg in range(4):  # split so the PE can start sooner
                nc.sync.dma_start(xT[:, 3 * kg:3 * (kg + 1), :],
                                  xT_d[:, 3 * kg:3 * (kg + 1), :])

            qkbias = persist.tile([128, 24], F32)
            nc.gpsimd.dma_start(qkbias[:], qkbias_d[:])
            wmask = persist.tile([128, 3, 256], BF16)
            nc.gpsimd.dma_start(wmask[:], wmask_d[:])
            ident = persist.tile([128, 128], BF16)
            nc.gpsimd.dma_start(ident[:], ident_d[:])

            QT = persist.tile([128, H, CHUNK], BF16)       # Q^T, s in [16,1040)
            KTw = persist.tile([128, H, NT * SLOT], BF16)  # K^T windows
            V = persist.tile([128, NT, H, VS], BF16)       # V natural + ones col
            Vtail = persist.tile([34, NT, H, VS], BF16)    # 32 tail rows + 2 glob
            Vglob = persist.tile([2, D], BF16)
            AT = persist.tile([128, H, CHUNK], BF16)       # attention out ^T

            nc.gpsimd.memset(V[:, :, :, 128:129], 1.0)
            nc.gpsimd.memset(Vtail[:, :, :, 128:129], 1.0)

            # ---------- phase 1: v projection (natural layout) ----------
            # Weight DMAs are staggered behind compute progress so startup
            # HBM bandwidth all goes to x^T (otherwise the PE idles ~20us).
            gate_insts = {}
            for fc in range(3):
                wv = wvp.tile([128, KO, 512], BF16, tag="wv")
                dma = nc.sync.dma_start(wv[:], wv_d[fc])
                if fc >= 1 and (fc - 1, 3) in gate_insts:
                    add_dep_helper(dma.ins, gate_insts[(fc - 1, 3)].ins,
                                   sync=True, reason="stagger wv dma")
                for st in range(9):
                    rows = 128 if st < 8 else 32
                    ps = proj_ps.tile([128, 512], F32, tag="pps")
                    for kt in range(KO):
                        nc.tensor.matmul(
                            ps[0:rows, :],
                            xT[:, kt, st * 128: st * 128 + rows], wv[:, kt, :],
                            start=(kt == 0), stop=(kt == KO - 1),
                        )
                    psv = ps.rearrange("p (h f) -> p h f", f=128)
                    if st < 8:
                        cp = nc.vector.tensor_copy(
                            V[:, st, 4 * fc:4 * fc + 4, 0:128], psv[:])
                        gate_insts[(fc, st)] = cp
                    if 1 <= st <= 8:
                        nc.vector.tensor_copy(
                            Vtail[0:32, st - 1, 4 * fc:4 * fc + 4, 0:128],
                            psv[0:32])
                    if st == 0:
                        nc.vector.tensor_copy(
                            Vglob[:, fc * 512:(fc + 1) * 512], ps[0:2, :])
            # replicate global v rows into every tail slot (partition shift -> DMA)
            vgv = Vglob.rearrange("p (h f) -> p h f", f=128)
            for t in range(NT):
                nc.sync.dma_start(Vtail[32:34, t, :, 0:128], vgv[:])

            # ---------- phase 2+3: per-head qk projection + attention ----------
            def qk_proj(h):
                # q section (f-tile h): own rows only, s in [16, 1040)
                w = wqp.tile([128, KO, 128], BF16, tag="wq")
                dma = nc.sync.dma_start(w[:], wqk_d[h])
                if h == 0:
                    add_dep_helper(dma.ins, gate_insts[(0, 6)].ins,
                                   sync=True, reason="stagger wq dma")
                for ncl in range(2):
                    ps = proj_ps.tile([128, 512], F32, tag="pps")
                    for kt in range(KO):
                        nc.tensor.matmul(
                            ps[:], w[:, kt, :],
                            xT[:, kt, OWN0 + ncl * 512: OWN0 + (ncl + 1) * 512],
                            start=(kt == 0), stop=(kt == KO - 1),
                        )
                    # QT = (ps + bias) * alpha, on DVE
                    nc.vector.tensor_scalar(
                        QT[:, h, ncl * 512:(ncl + 1) * 512], ps[:],
                        qkbias[:, h:h + 1], ALPHA,
                        mybir.AluOpType.add, mybir.AluOpType.mult,
                    )
                # k section (f-tile 12+h): full extended rows, windowed layout
                ft = 12 + h
                w2 = wqp.tile([128, KO, 128], BF16, tag="wq")
                nc.sync.dma_start(w2[:], wqk_d[ft])
                ktw = KTw[:, h, :].rearrange("p (t j) -> p t j", j=SLOT)
                bias = qkbias[:, ft:ft + 1]
                ps3 = p3_ps.tile([128, 32], F32, tag="p3")
                for ncl in range(2):
                    ps = proj_ps.tile([128, 512], F32, tag="pps")
                    for kt in range(KO):
                        nc.tensor.matmul(
                            ps[:], w2[:, kt, :],
                            xT[:, kt, ncl * 512:(ncl + 1) * 512],
                            start=(kt == 0), stop=(kt == KO - 1),
                        )
                        if ncl == 1:
                            # keys 1024..1056: same weights — its LDWEIGHTS
                            # hides under the 512-wide stream above
                            nc.tensor.matmul(
                                ps3[:], w2[:, kt, :], xT[:, kt, 1024:1056],
                                start=(kt == 0), stop=(kt == KO - 1),
                            )
                    psv = ps.rearrange("p (t j) -> p t j", j=128)
                    t0 = 4 * ncl
                    nc.scalar.activation(
                        ktw[:, t0:t0 + 4, 0:128], psv[:, 0:4, :],
                        ident_fn, bias=bias)
                    if ncl == 0:
                        nc.scalar.activation(
                            ktw[:, 0:3, 128:160], psv[:, 1:4, 0:32],
                            ident_fn, bias=bias)
                        nc.scalar.activation(
                            ktw[:, 0:NT, 160:162],
                            ps[:, None, 0:2].to_broadcast([128, NT, 2]),
                            ident_fn, bias=bias)
                    else:
                        nc.scalar.activation(
                            ktw[:, 3:7, 128:160], psv[:, 0:4, 0:32],
                            ident_fn, bias=bias)
                        nc.scalar.activation(
                            ktw[:, 7:8, 128:160], ps3[:, None, 0:32],
                            ident_fn, bias=bias)

            def attention(h):
                ktw = KTw[:, h, :].rearrange("p (t j) -> p t j", j=SLOT)
                for t in range(NT):
                    m = 0 if t == 0 else (2 if t == NT - 1 else 1)
                    qt = QT[:, h, t * 128:(t + 1) * 128]
                    sc = sc_ps.tile([128, 256], F32, tag="sc")
                    nc.tensor.matmul(sc[:, 0:128], ktw[:, t, 0:128], qt,
                                     start=True, stop=True)
                    nc.tensor.matmul(sc[0:34, 128:256], ktw[:, t, 128:162], qt,
                                     start=True, stop=True)
                    pe = work.tile([128, 256], BF16, tag="pe")
                    nc.scalar.activation(pe[:], sc[:], exp_fn)
                    pm = work.tile([128, 256], BF16, tag="pm")
                    nc.vector.tensor_mul(pm[:], pe[:], wmask[:, m, :])

                    ad = ad_ps.tile([128, 260], F32, tag="ad")
                    nc.tensor.matmul(ad[:, 0:VS], pm[:, 0:128],
                                     V[:, t, h, :], start=True, stop=False)
                    nc.tensor.matmul(ad[:, 0:VS], pm[0:34, 128:256],
                                     Vtail[:, t, h, :], start=False, stop=True)

                    r = work.tile([128, 1], F32, tag="r")
                    nc.vector.reciprocal(r[:], ad[:, 128:129])
                    a_sb = work.tile([128, 128], BF16, tag="a_sb")
                    nc.vector.tensor_scalar_mul(a_sb[:], ad[:, 0:128], r[:])
                    # transpose: A^T = a_sb.T @ I
                    nc.tensor.matmul(ad[:, 132:260], a_sb[:], ident[:],
                                     start=True, stop=True)
                    nc.vector.tensor_copy(AT[:, h, t * 128:(t + 1) * 128],
                                          ad[:, 132:260])

            qk_proj(0)
            for h in range(1, H):
                qk_proj(h)
                attention(h - 1)
            attention(H - 1)

            # ---------- phase 4: out projection ----------
            for ft in range(12):
                wo = wop.tile([128, KO, 128], BF16, tag="wo")
                nc.sync.dma_start(wo[:], wo_d[ft])
                for ncl in range(2):
                    ps = proj_ps.tile([128, 512], F32, tag="pps")
                    for kt in range(KO):
                        nc.tensor.matmul(
                            ps[:], wo[:, kt, :], AT[:, kt, ncl * 512:(ncl + 1) * 512],
                            start=(kt == 0), stop=(kt == KO - 1),
                        )
                    y = yst.tile([128, 512], F32, tag="y")
                    nc.scalar.copy(y[:], ps[:])
                    nc.sync.dma_start(
                        yT_d.rearrange("(fo p) s -> p fo s", p=128)
                        [:, ft, ncl * 512:(ncl + 1) * 512], y[:])

    nc.compile()
    _prog_cache["nc"] = nc
    return nc


def _host_prep(x, qkv_w, qkv_b, out_w, out_b, group_scale):
    """Build the per-core input maps (numpy only)."""
    bf16 = ml_dtypes.bfloat16
    g = np.asarray(group_scale, np.float64)
    e = np.exp(g - g.max())
    gw = (e / e.sum()).astype(np.float64)

    KO = D // 128
    wT = qkv_w.astype(np.float32).T              # [D, 3D]
    # q/k sections, f-tile major: [24, 128, KO, 128]
    wqk = np.ascontiguousarray(
        wT[:, :2 * D].reshape(KO, 128, 24, 128).transpose(2, 1, 0, 3)
    ).astype(bf16)
    # v section, 512-wide f-chunk major: [3, 128, KO, 512]
    wv = np.ascontiguousarray(
        wT[:, 2 * D:].reshape(KO, 128, 3, 512).transpose(2, 1, 0, 3)
    ).astype(bf16)
    woT = out_w.astype(np.float32).T             # [D, D]
    wo = np.ascontiguousarray(
        woT.reshape(KO, 128, 12, 128).transpose(2, 1, 0, 3)
    ).astype(bf16)

    qkbias = np.zeros((128, 24), np.float32)
    for ft in range(24):
        qkbias[:, ft] = qkv_b[ft * 128:(ft + 1) * 128].astype(np.float32)

    ident = np.eye(128, dtype=bf16)

    band = [(-1, 0), (1, 0), (-2, 0), (2, 0), (-3, 0), (3, 0),
            (-5, 1), (5, 1), (-10, 1), (10, 1)]

    in_maps = []
    for core in range(N_CORES):
        b, chunk = divmod(core, 4)
        c0 = chunk * CHUNK
        xe = np.zeros((XROWS, D), np.float32)
        xe[0] = x[b, 0]
        xe[1] = x[b, S - 1]
        if chunk > 0:
            xe[2:16] = x[b, c0 - 14:c0]
        xe[16:16 + CHUNK] = x[b, c0:c0 + CHUNK]
        if chunk < 3:
            xe[16 + CHUNK:26 + CHUNK] = x[b, c0 + CHUNK:c0 + CHUNK + 10]
        xT = np.ascontiguousarray(
            xe.T.reshape(KO, 128, XROWS).transpose(1, 0, 2)).astype(bf16)

        # combined banded weight mask, [j, slot, 256]:
        #   cols 0:128 -> window piece a (keys 128t..128t+128)
        #   cols 128:256 rows 0:32 -> tail keys, rows 32:34 -> globals
        wm = np.zeros((128, 3, 256), np.float64)
        for slot, t in ((0, 0), (1, 3), (2, NT - 1)):
            for p in range(128):
                s = c0 + 128 * t + p
                for off, grp in band:
                    a = min(max(s + off, 0), S - 1)
                    j = (a - c0 + 16) - 128 * t
                    if j < 128:
                        wm[j, slot, p] += gw[grp]
                    else:
                        wm[j - 128, slot, 128 + p] += gw[grp]
            wm[32, slot, 128:256] += gw[2]
            wm[33, slot, 128:256] += gw[2]

        in_maps.append({
            "xT": xT,
            "wqk": wqk,
            "wv": wv,
            "wo": wo,
            "qkbias": qkbias,
            "wmask": wm.astype(bf16),
            "ident": ident,
        })

    y_const = (qkv_b[2 * D:3 * D].astype(np.float64) @
               out_w.astype(np.float64).T + out_b.astype(np.float64)
               ).astype(np.float32)
    return in_maps, y_const


def kernel(x, qkv_w, qkv_b, out_w, out_b, group_scale, _run_kwargs=None):
    x = np.asarray(x)
    in_maps, y_const = _host_prep(
        np.asarray(x, np.float32), np.asarray(qkv_w, np.float32),
        np.asarray(qkv_b, np.float32), np.asarray(out_w, np.float32),
        np.asarray(out_b, np.float32), np.asarray(group_scale, np.float32))
    nc = _build_program()
    kwargs = _run_kwargs or {}
    res = run_bass_kernel_spmd(nc, in_maps, core_ids=list(range(N_CORES)), **kwargs)
    out = np.empty((B, S, D), np.float32)
    for core in range(N_CORES):
        b, chunk = divmod(core, 4)
        r = res.results[core]
        yT = r["yT"] if isinstance(r, dict) else r
        out[b, chunk * CHUNK:(chunk + 1) * CHUNK] = np.asarray(yT, np.float32).T
    out += y_const
    if kwargs.get("trace"):
        kernel.last_exec_time_ns = res.exec_time_ns
    return out


if __name__ == "__main__":
    rng = np.random.default_rng(0)
    x = rng.standard_normal((B, S, D), dtype=np.float32)
    qkv_w = (rng.standard_normal((3 * D, D), dtype=np.float32) / np.sqrt(D))
    qkv_b = rng.standard_normal(3 * D, dtype=np.float32) * 0.01
    out_w = rng.standard_normal((D, D), dtype=np.float32) / np.sqrt(D)
    out_b = rng.standard_normal(D, dtype=np.float32) * 0.01
    gs = rng.standard_normal(3, dtype=np.float32)
    y = kernel(x=x, qkv_w=qkv_w, qkv_b=qkv_b, out_w=out_w, out_b=out_b,
               group_scale=gs)
    print("ok", y.shape, float(np.abs(y).mean()))
